# revision 11
# baseline (speedup 1.0000x reference)
"""AlphaFold-style gated attention (pair bias + sigmoid gating) on 8 Trainium2
NeuronCores.

Problem shapes (hardcoded): B=4, Q=K=1024, C=256, H=8, D=32, fp32.

Sharding: (batch x head-group) -> core = b*2 + hg; each core handles 1 batch
and 4 heads.  Each core computes a partial output [Q, C] (its 4 heads pushed
through the output projection); the host sums the two partials per batch.
bias `bo` is folded into the head-group-0 core's partial.

Per-core device kernel, fp16 matmul datapath (fp32 PSUM accumulation):
  qT/kT/gT = W @ x.T                [HD=128, Q]  (q scale folded into Wq)
  v        = kv_x @ Wv.T            [K-tile, HD] x 8 tiles
  S.T      = k_h^T-block @ q_T      [K-tile, Q]  per head, PSUM fp32
  es       = exp(S.T + (mask - SHIFT))   ACT, per-partition bias, fp16 out
  e2       = es * exp_pair.T        DVE f16 2x mode (pair bias folded in via
                                    exp(S+pair) = exp(S)*exp(pair); exp(pair)
                                    precomputed on host, fp16 in DRAM)
  o.T     += v_h.T @ e2             col-packed 4 heads -> [128, Q] PSUM
  rowsum  += ones @ e2              col-packed, M=32 dup rows -> recipB
  o_eff    = o.T * g.T * recipB     DVE (fp16 out)
  out      = o_eff.T @ Wo.T + bo    natural [Q, C], fp16 matmul

The exp SHIFT keeps es/e2 within fp16 range; it cancels in o/rowsum.
The identity-matmul pair accumulation of the earlier version (64 extra
128x128x512 matmuls, ~20us PE) is replaced by the DVE multiply, which
rides in the vector engine's idle time.  All DMAs go on the two HWDGE
rings (sync + scalar); gpsimd is unused so its expensive dge_drain
disappears from the tail.
"""

import math

import numpy as np

B, Q, K, C, H, D = 4, 1024, 1024, 256, 8, 32
HPG = 4  # heads per group
HG = 2  # head groups
NCORES = 8
KT = K // 128  # 8 K-tiles
SHIFT = 2.0  # exp shift (softmax-invariant), keeps es*ep < fp16 max

PAIR_BUFS = 32
ES_BUFS = 6
E2_BUFS = 8


def _build_program():
    import concourse.bass as bass
    import concourse.tile as tile
    from concourse import bacc, mybir

    f32 = mybir.dt.float32
    f16 = mybir.dt.float16
    AF = mybir.ActivationFunctionType

    nc = bacc.Bacc("TRN2", target_bir_lowering=False, debug=False)

    # ---- I/O (host-prepped layouts, see _shard_inputs) ----------------
    # Every dma_start costs ~0.6us on the HWDGE ring regardless of size, so
    # the small tensors are packed into three transfers:
    #   w4   f16 [128, 1024] = wq | wk | wv | wg
    #   sm16 f16 [128, 288]  = ones | wo
    #   sm32 f32 [128, 265]  = bg | mask | bo
    d_qx = nc.dram_tensor("qx", [128, 2 * Q], f16, kind="ExternalInput").ap()
    d_kvx = nc.dram_tensor("kvx", [128, 2 * K], f16, kind="ExternalInput").ap()
    d_ep = nc.dram_tensor("ep", [2, KT, 2, 128, Q], f16, kind="ExternalInput").ap()
    d_w4 = nc.dram_tensor("w4", [128, 1024], f16, kind="ExternalInput").ap()
    d_sm16 = nc.dram_tensor("sm16", [128, 288], f16, kind="ExternalInput").ap()
    d_sm32 = nc.dram_tensor("sm32", [128, 265], f32, kind="ExternalInput").ap()
    d_out = nc.dram_tensor("out", [Q, C], f32, kind="ExternalOutput").ap()

    with tile.TileContext(nc) as tc:
        from contextlib import ExitStack

        with ExitStack() as ctx:
            cp = ctx.enter_context(tc.tile_pool(name="consts", bufs=1))
            act_p = ctx.enter_context(tc.tile_pool(name="acts", bufs=1))
            pair_p = ctx.enter_context(tc.tile_pool(name="pair", bufs=PAIR_BUFS))
            es_p = ctx.enter_context(tc.tile_pool(name="es", bufs=ES_BUFS))
            e2_p = ctx.enter_context(tc.tile_pool(name="e2", bufs=E2_BUFS))
            nrm_p = ctx.enter_context(tc.tile_pool(name="nrm", bufs=4))
            mid_p = ctx.enter_context(tc.tile_pool(name="mid", bufs=1))
            out_p = ctx.enter_context(tc.tile_pool(name="outs", bufs=3))

            w4 = cp.tile([128, 1024], f16)
            sm16 = cp.tile([128, 288], f16)
            sm32 = cp.tile([128, 265], f32)
            wq = w4[:, 0:256]
            wk = w4[:, 256:512]
            wv = w4[:, 512:768]
            wg = w4[:, 768:1024]
            ones = sm16[:, 0:32]
            wo = sm16[:, 32:288]
            bg = sm32[:, 0:1]
            mask = sm32[:, 1 : 1 + KT]
            bo = sm32[:, 1 + KT : 1 + KT + 256]
            qx = act_p.tile([128, 2 * Q], f16)
            kvx = act_p.tile([128, 2 * K], f16)

            # preload the ACT Exp/Sigmoid tables off the critical path: a
            # dummy activation on a memset scratch during the DMA window.
            scr0 = cp.tile([128, 1], f32)
            scr1 = cp.tile([128, 1], f32)
            nc.vector.memset(scr0[:], 0.0)
            nc.scalar.activation(scr1[:], scr0[:], AF.Exp)
            nc.scalar.activation(scr1[:], scr0[:], AF.Sigmoid)

            # input DMAs on the scalar HWDGE ring: the sync ring streams the
            # 8MB of exp(pair) tiles back-to-back, and completion receipts
            # there lag several transfers behind; the near-idle scalar ring
            # signals completion promptly so phase 1 can start early.
            nc.scalar.dma_start(qx[:], d_qx[:])
            nc.scalar.dma_start(w4[:], d_w4[:])
            nc.scalar.dma_start(kvx[:], d_kvx[:])
            nc.scalar.dma_start(sm16[:], d_sm16[:])
            nc.scalar.dma_start(sm32[:], d_sm32[:])

            # issue all exp(pair) DMAs up-front (pool slots throttle them in
            # order); tile (qh, kc, hp2) = heads (2*hp2, 2*hp2+1) side by
            # side for q-half qh -- matches the es tile layout exactly.
            pair_t = {}
            for kc in range(KT):
                for qh in range(2):
                    for hp2 in range(2):
                        t = pair_p.tile(
                            [128, Q], f16, tag="pair", name=f"ep_{qh}_{kc}_{hp2}"
                        )
                        nc.sync.dma_start(t[:], d_ep[qh, kc, hp2])
                        pair_t[(qh, kc, hp2)] = t

            q_sb = mid_p.tile([128, Q], f16)
            k_sb = mid_p.tile([128, K], f16)
            g_sb = mid_p.tile([128, Q], f32)
            v_sb = [
                mid_p.tile([128, 128], f16, tag=f"v{i}", name=f"v{i}")
                for i in range(KT)
            ]

            # ---- phase 1: projections ------------------------------------
            # emission order: q/k halves first (QK(0,0) only needs qh=0
            # halves), v tiles next, sigmoid-gate last (not on the critical
            # path until normalize).
            with tc.tile_pool(name="ps1", bufs=2, space="PSUM") as ps1:

                def proj(w_sb, x_sb, dst, qh, func=None, bias=None):
                    ps = ps1.tile([128, 512], f32, tag="proj", name="ps_proj")
                    for j in range(2):
                        nc.tensor.matmul(
                            ps[:],
                            w_sb[:, bass.ts(j, 128)],
                            x_sb[:, j * Q + qh * 512 :][:, :512],
                            start=(j == 0),
                            stop=(j == 1),
                        )
                    if func is None:
                        nc.vector.tensor_copy(dst[:, bass.ts(qh, 512)], ps[:])
                    else:
                        nc.scalar.activation(
                            dst[:, bass.ts(qh, 512)], ps[:], func, bias=bias
                        )

                proj(wq, qx, q_sb, 0)
                proj(wk, kvx, k_sb, 0)
                proj(wq, qx, q_sb, 1)
                proj(wk, kvx, k_sb, 1)
                for kt in range(KT):
                    ps = ps1.tile([128, 128], f32, tag="vproj", name="ps_vproj")
                    for j in range(2):
                        nc.tensor.matmul(
                            ps[:],
                            kvx[:, j * K + kt * 128 :][:, :128],
                            wv[:, bass.ts(j, 128)],
                            start=(j == 0),
                            stop=(j == 1),
                        )
                    nc.vector.tensor_copy(v_sb[kt][:], ps[:])
                proj(wg, qx, g_sb, 0, AF.Sigmoid, bg)
                proj(wg, qx, g_sb, 1, AF.Sigmoid, bg)

            # ---- phase 2/3/4: attention, software-pipelined emission -----
            # All 32 exp(pair) chunks stay resident in SBUF.  Engine queues
            # execute in scheduled (~program) order, so emission order is
            # the software pipeline: QK/exp/mul of unit u are emitted one
            # step ahead of AV/rowsum of unit u-1, keeping ACT fed; sweep
            # 0's normalization/output-projection is deferred into sweep
            # 1's early units so its DVE/PE work hides under sweep-1
            # compute instead of stalling the chain.
            # PSUM: 3 S tiles (6 banks) + (o,r) accumulators (2 banks) = 8.
            with (
                tc.tile_pool(name="ps_s", bufs=3, space="PSUM") as ps_s,
                tc.tile_pool(name="ps_o", bufs=1, space="PSUM") as ps_o,
                tc.tile_pool(name="ps_r", bufs=1, space="PSUM") as ps_r,
            ):
                o_eff = mid_p.tile([128, Q], f16)
                o_ps = {}
                r_ps = {}
                e2_t = {}

                def qk_exp_mul(qh, kc):
                    sp = [
                        ps_s.tile(
                            [128, 1024], f32, tag="s", name=f"sp_{kc}_{qh}_{hp2}"
                        )
                        for hp2 in range(2)
                    ]
                    for h in range(HPG):
                        hp = slice(32 * h, 32 * h + 32)
                        nc.tensor.matmul(
                            sp[h // 2][:, bass.ts(h % 2, 512)],
                            k_sb[hp, bass.ts(kc, 128)],
                            q_sb[hp, bass.ts(qh, 512)],
                            start=True,
                            stop=True,
                            tile_position=(32 * h, 0),
                            skip_group_check=True,
                        )
                    e2s = []
                    for hp2 in range(2):
                        es = es_p.tile(
                            [128, 1024], f16, tag="es", name=f"es_{kc}_{qh}_{hp2}"
                        )
                        nc.scalar.activation(
                            es[:], sp[hp2][:], AF.Exp, bias=mask[:, kc : kc + 1]
                        )
                        e2 = e2_p.tile(
                            [128, 1024], f16, tag="e2", name=f"e2_{kc}_{qh}_{hp2}"
                        )
                        nc.vector.tensor_mul(e2[:], es[:], pair_t[(qh, kc, hp2)][:])
                        e2s.append(e2)
                    e2_t[(qh, kc)] = e2s

                def av_rowsum(qh, kc):
                    if kc == 0:
                        o_ps[qh] = ps_o.tile(
                            [128, 512], f32, tag="o", name=f"o_ps{qh}"
                        )
                        r_ps[qh] = ps_r.tile(
                            [128, 512], f32, tag="r", name=f"r_ps{qh}"
                        )
                    e2s = e2_t.pop((qh, kc))
                    for h in range(HPG):
                        hp = slice(32 * h, 32 * h + 32)
                        nc.tensor.matmul(
                            o_ps[qh][hp, :],
                            v_sb[kc][:, hp],
                            e2s[h // 2][:, bass.ts(h % 2, 512)],
                            start=(kc == 0),
                            stop=(kc == KT - 1),
                            tile_position=(0, 32 * h),
                            skip_group_check=True,
                        )
                    for h in range(HPG):
                        hp = slice(32 * h, 32 * h + 32)
                        nc.tensor.matmul(
                            r_ps[qh][hp, :],
                            ones[:],
                            e2s[h // 2][:, bass.ts(h % 2, 512)],
                            start=(kc == 0),
                            stop=(kc == KT - 1),
                            tile_position=(0, 32 * h),
                            skip_group_check=True,
                        )

                def normalize(qh):
                    recip = nrm_p.tile([128, 512], f32, tag="recip", name="recip")
                    rscr = nrm_p.tile([128, 512], f32, tag="rscr", name="rscr")
                    nc.vector.reciprocal_approx_accurate(
                        recip[:], r_ps[qh][:], rscr[:]
                    )
                    geff = nrm_p.tile([128, 512], f32, tag="geff", name="geff")
                    nc.vector.tensor_mul(geff[:], g_sb[:, bass.ts(qh, 512)], recip[:])
                    nc.vector.tensor_mul(
                        o_eff[:, bass.ts(qh, 512)], o_ps[qh][:], geff[:]
                    )

                def outproj(qh):
                    for qt in range(4 * qh, 4 * qh + 4):
                        ps = ps_s.tile([128, 256], f32, tag="s", name="ps_out")
                        nc.tensor.matmul(
                            ps[:],
                            o_eff[:, bass.ts(qt, 128)],
                            wo[:],
                            start=True,
                            stop=True,
                        )
                        ot = out_p.tile([128, 256], f32, tag="ot", name="ot")
                        nc.vector.tensor_add(ot[:], ps[:], bo[:])
                        nc.sync.dma_start(d_out[bass.ts(qt, 128), :], ot[:])

                # sweep 0
                for kc in range(KT):
                    qk_exp_mul(0, kc)
                    if kc > 0:
                        av_rowsum(0, kc - 1)
                # sweep 1, with sweep 0's drain work folded in
                for kc in range(KT):
                    qk_exp_mul(1, kc)
                    if kc == 0:
                        av_rowsum(0, KT - 1)
                    elif kc == 1:
                        normalize(0)  # frees o/r banks before av_rowsum(1,0)
                        av_rowsum(1, 0)
                    else:
                        if kc == 3:
                            outproj(0)
                        av_rowsum(1, kc - 1)
                av_rowsum(1, KT - 1)
                normalize(1)
                outproj(1)

    nc.compile()
    return nc


_NC_CACHE = None


def _get_program():
    global _NC_CACHE
    if _NC_CACHE is None:
        _NC_CACHE = _build_program()
    return _NC_CACHE


def _shard_inputs(q_x, kv_x, bias_mask, bias_pair, Wq, Wk, Wv, Wo, bo, Wg, bg):
    """Build the 8 per-core input maps."""
    f = np.float32
    f16 = np.float16
    scale = 1.0 / math.sqrt(D)

    def fold2(w_t):  # [256, M] -> [128, 2*M] sbuf layout
        return np.ascontiguousarray(
            w_t.reshape(2, 128, w_t.shape[1]).transpose(1, 0, 2).reshape(128, -1)
        )

    in_maps = []
    for core in range(NCORES):
        b, hg = core // HG, core % HG
        hs = slice(hg * 128, hg * 128 + 128)  # H*D slice for this head group
        qxT = np.ascontiguousarray(q_x[b].T).astype(f)  # [256, 1024]
        kvxT = np.ascontiguousarray(kv_x[b].T).astype(f)
        # exp(pair) tiles: [qh, kc, hp2, 128, Q]; tile (qh,kc,hp2) holds heads
        # (2*hp2, 2*hp2+1) side by side for q-half qh, k-tile kc (transposed
        # to [k, q] to match the S.T layout).
        epT = np.exp(
            bias_pair[b, hg * HPG : hg * HPG + HPG].astype(f)
        ).transpose(0, 2, 1)  # [4, K, Q]
        ep = (
            epT.reshape(HPG, KT, 128, 2, 512)
            .transpose(3, 1, 0, 2, 4)  # [qh, kc, h, 128, 512]
            .reshape(2, KT, 2, 2, 128, 512)
            .transpose(0, 1, 2, 4, 3, 5)  # [qh, kc, hp2, 128, hi, 512]
            .reshape(2, KT, 2, 128, Q)
        )
        w4 = np.concatenate(
            [
                fold2(np.ascontiguousarray(Wq[hs].T) * scale),
                fold2(np.ascontiguousarray(Wk[hs].T)),
                fold2(np.ascontiguousarray(Wv[hs].T)),
                fold2(np.ascontiguousarray(Wg[hs].T)),
            ],
            axis=1,
        )
        sm16 = np.concatenate(
            [np.ones((128, 32), f), np.ascontiguousarray(Wo[:, hs].T)], axis=1
        )
        bo_b = (
            np.broadcast_to(bo, (128, C)).astype(f)
            if hg == 0
            else np.zeros((128, C), f)
        )
        sm32 = np.concatenate(
            [
                np.ascontiguousarray(bg[hs].reshape(128, 1)).astype(f),
                np.ascontiguousarray(
                    bias_mask[b, 0, 0].reshape(KT, 128).T - SHIFT
                ).astype(f),
                bo_b,
            ],
            axis=1,
        )
        m16 = {
            "qx": fold2(qxT),
            "kvx": fold2(kvxT),
            "ep": ep,
            "w4": w4,
            "sm16": sm16,
        }
        m = {k: np.ascontiguousarray(v, f16) for k, v in m16.items()}
        m["sm32"] = np.ascontiguousarray(sm32, f)
        in_maps.append(m)
    return in_maps


def run_on_cores(in_maps, trace=False, trace_kwargs={}):
    from concourse.bass_utils import run_bass_kernel_spmd

    nc = _get_program()
    return run_bass_kernel_spmd(
        nc, in_maps, list(range(NCORES)), trace=trace, trace_kwargs=trace_kwargs
    )


def kernel(q_x, kv_x, bias_mask, bias_pair, Wq, Wk, Wv, Wo, bo, Wg, bg):
    in_maps = _shard_inputs(
        q_x, kv_x, bias_mask, bias_pair, Wq, Wk, Wv, Wo, bo, Wg, bg
    )
    res = run_on_cores(in_maps).results
    out = np.empty((B, Q, C), np.float32)
    for b in range(B):
        out[b] = res[b * HG + 0]["out"] + res[b * HG + 1]["out"]
    return out


# revision 21
# speedup vs baseline: 1.0693x; 1.0693x over previous
"""AlphaFold-style gated attention (pair bias + sigmoid gating) on 8 Trainium2
NeuronCores.

Problem shapes (hardcoded): B=4, Q=K=1024, C=256, H=8, D=32, fp32.

Sharding: (batch x head-group) -> core = b*2 + hg; each core handles 1 batch
and 4 heads.  Each core computes a partial output [Q, C] (its 4 heads pushed
through the output projection); the host sums the two partials per batch.
bias `bo` is folded into the head-group-0 core's partial.

Per-core device kernel, fp16 matmul datapath (fp32 PSUM accumulation):
  qT/kT/gT = W @ x.T                [HD=128, Q]  (q scale folded into Wq)
  v        = kv_x @ Wv.T            [K-tile, HD] x 8 tiles
  S.T      = k_h^T-block @ q_T      [K-tile, Q]  per head, PSUM fp32
  es       = exp(S.T + (mask - SHIFT))   ACT, per-partition bias, fp16 out
  e2       = es * exp_pair.T        DVE f16 2x mode (pair bias folded in via
                                    exp(S+pair) = exp(S)*exp(pair); exp(pair)
                                    precomputed on host, fp16 in DRAM)
  o.T     += v_h.T @ e2             col-packed 4 heads -> [128, Q] PSUM
  rowsum  += ones @ e2              col-packed, M=32 dup rows -> recipB
  o_eff    = o.T * g.T * recipB     DVE (fp16 out)
  out      = o_eff.T @ Wo.T + bo    natural [Q, C], fp16 matmul

The exp SHIFT keeps es/e2 within fp16 range; it cancels in o/rowsum.
The identity-matmul pair accumulation of the earlier version (64 extra
128x128x512 matmuls, ~20us PE) is replaced by the DVE multiply, which
rides in the vector engine's idle time.  All DMAs go on the two HWDGE
rings (sync + scalar); gpsimd is unused so its expensive dge_drain
disappears from the tail.
"""

import math

import numpy as np

B, Q, K, C, H, D = 4, 1024, 1024, 256, 8, 32
HPG = 4  # heads per group
HG = 2  # head groups
NCORES = 8
KT = K // 128  # 8 K-tiles
SHIFT = 2.0  # exp shift (softmax-invariant), keeps es*ep < fp16 max

PAIR_BUFS = 32
ES_BUFS = 6
E2_BUFS = 12


def _build_program():
    import concourse.bass as bass
    import concourse.tile as tile
    from concourse import bacc, mybir

    f32 = mybir.dt.float32
    f16 = mybir.dt.float16
    AF = mybir.ActivationFunctionType

    nc = bacc.Bacc("TRN2", target_bir_lowering=False, debug=False)

    # ---- I/O (host-prepped layouts, see _shard_inputs) ----------------
    # Every dma_start costs ~0.6us on the HWDGE ring regardless of size, so
    # the small tensors are packed into three transfers:
    #   w4   f16 [128, 1024] = wq | wk | wv | wg
    #   sm16 f16 [128, 288]  = ones | wo
    #   sm32 f32 [128, 265]  = bg | mask | bo
    d_qx = nc.dram_tensor("qx", [128, 2 * Q], f16, kind="ExternalInput").ap()
    d_kvx = nc.dram_tensor("kvx", [128, 2 * K], f16, kind="ExternalInput").ap()
    d_ep = nc.dram_tensor("ep", [2, KT, 2, 128, Q], f16, kind="ExternalInput").ap()
    d_w4 = nc.dram_tensor("w4", [128, 1024], f16, kind="ExternalInput").ap()
    d_sm16 = nc.dram_tensor("sm16", [128, 288], f16, kind="ExternalInput").ap()
    d_sm32 = nc.dram_tensor("sm32", [128, 265], f32, kind="ExternalInput").ap()
    d_out = nc.dram_tensor("out", [Q, C], f32, kind="ExternalOutput").ap()

    with tile.TileContext(nc) as tc:
        from contextlib import ExitStack

        with ExitStack() as ctx:
            cp = ctx.enter_context(tc.tile_pool(name="consts", bufs=1))
            act_p = ctx.enter_context(tc.tile_pool(name="acts", bufs=1))
            pair_p = ctx.enter_context(tc.tile_pool(name="pair", bufs=PAIR_BUFS))
            es_p = ctx.enter_context(tc.tile_pool(name="es", bufs=ES_BUFS))
            e2_p = ctx.enter_context(tc.tile_pool(name="e2", bufs=E2_BUFS))
            nrm_p = ctx.enter_context(tc.tile_pool(name="nrm", bufs=4))
            mid_p = ctx.enter_context(tc.tile_pool(name="mid", bufs=1))
            out_p = ctx.enter_context(tc.tile_pool(name="outs", bufs=3))

            w4 = cp.tile([128, 1024], f16)
            sm16 = cp.tile([128, 288], f16)
            sm32 = cp.tile([128, 265], f32)
            wq = w4[:, 0:256]
            wk = w4[:, 256:512]
            wv = w4[:, 512:768]
            wg = w4[:, 768:1024]
            ones = sm16[:, 0:32]
            wo = sm16[:, 32:288]
            bg = sm32[:, 0:1]
            mask = sm32[:, 1 : 1 + KT]
            bo = sm32[:, 1 + KT : 1 + KT + 256]
            qx = act_p.tile([128, 2 * Q], f16)
            kvx = act_p.tile([128, 2 * K], f16)

            # input DMAs on the scalar HWDGE ring: the sync ring streams the
            # 8MB of exp(pair) tiles back-to-back, and completion receipts
            # there lag several transfers behind; the near-idle scalar ring
            # signals completion promptly so phase 1 can start early.
            nc.scalar.dma_start(qx[:], d_qx[:])
            nc.scalar.dma_start(w4[:], d_w4[:])
            nc.scalar.dma_start(kvx[:], d_kvx[:])
            nc.scalar.dma_start(sm16[:], d_sm16[:])
            nc.scalar.dma_start(sm32[:], d_sm32[:])

            # preload the ACT Exp/Sigmoid tables off the critical path: a
            # dummy activation on a memset scratch during the DMA window.
            scr0 = cp.tile([128, 1], f32)
            scr1 = cp.tile([128, 1], f32)
            nc.vector.memset(scr0[:], 0.0)
            nc.scalar.activation(scr1[:], scr0[:], AF.Exp)
            nc.scalar.activation(scr1[:], scr0[:], AF.Sigmoid)

            # issue all exp(pair) DMAs up-front (pool slots throttle them in
            # order); tile (qh, kc, hp2) = heads (2*hp2, 2*hp2+1) side by
            # side for q-half qh -- matches the es tile layout exactly.
            pair_t = {}
            for kc in range(KT):
                for qh in range(2):
                    for hp2 in range(2):
                        t = pair_p.tile(
                            [128, Q], f16, tag="pair", name=f"ep_{qh}_{kc}_{hp2}"
                        )
                        nc.sync.dma_start(t[:], d_ep[qh, kc, hp2])
                        pair_t[(qh, kc, hp2)] = t

            q_sb = mid_p.tile([128, Q], f16)
            k_sb = mid_p.tile([128, K], f16)
            g_sb = mid_p.tile([128, Q], f32)
            v_sb = [
                mid_p.tile([128, 128], f16, tag=f"v{i}", name=f"v{i}")
                for i in range(KT)
            ]

            # ---- phase 1: projections ------------------------------------
            # emission order: q/k halves first (QK(0,0) only needs qh=0
            # halves), v tiles next, sigmoid-gate last (not on the critical
            # path until normalize).
            with tc.tile_pool(name="ps1", bufs=2, space="PSUM") as ps1:

                def proj(w_sb, x_sb, dst, qh, func=None, bias=None):
                    ps = ps1.tile([128, 512], f32, tag="proj", name="ps_proj")
                    for j in range(2):
                        nc.tensor.matmul(
                            ps[:],
                            w_sb[:, bass.ts(j, 128)],
                            x_sb[:, j * Q + qh * 512 :][:, :512],
                            start=(j == 0),
                            stop=(j == 1),
                        )
                    if func is None:
                        nc.vector.tensor_copy(dst[:, bass.ts(qh, 512)], ps[:])
                    else:
                        nc.scalar.activation(
                            dst[:, bass.ts(qh, 512)], ps[:], func, bias=bias
                        )

                proj(wq, qx, q_sb, 0)
                proj(wk, kvx, k_sb, 0)
                proj(wq, qx, q_sb, 1)
                proj(wk, kvx, k_sb, 1)
                for kt in range(KT):
                    ps = ps1.tile([128, 128], f32, tag="vproj", name="ps_vproj")
                    for j in range(2):
                        nc.tensor.matmul(
                            ps[:],
                            kvx[:, j * K + kt * 128 :][:, :128],
                            wv[:, bass.ts(j, 128)],
                            start=(j == 0),
                            stop=(j == 1),
                        )
                    nc.vector.tensor_copy(v_sb[kt][:], ps[:])
                proj(wg, qx, g_sb, 0, AF.Sigmoid, bg)
                proj(wg, qx, g_sb, 1, AF.Sigmoid, bg)

            # ---- phase 2/3/4: attention, software-pipelined emission -----
            # All 32 exp(pair) chunks stay resident in SBUF.  Engine queues
            # execute in scheduled (~program) order, so emission order is
            # the software pipeline: QK/exp/mul of unit u are emitted one
            # step ahead of AV/rowsum of unit u-1, keeping ACT fed; sweep
            # 0's normalization/output-projection is deferred into sweep
            # 1's early units so its DVE/PE work hides under sweep-1
            # compute instead of stalling the chain.
            # PSUM: 3 S tiles (6 banks) + (o,r) accumulators (2 banks) = 8.
            with (
                tc.tile_pool(name="ps_s", bufs=3, space="PSUM") as ps_s,
                tc.tile_pool(name="ps_o", bufs=1, space="PSUM") as ps_o,
                tc.tile_pool(name="ps_r", bufs=1, space="PSUM") as ps_r,
            ):
                o_eff = mid_p.tile([128, Q], f16)
                o_ps = {}
                r_ps = {}
                e2_t = {}

                def qk_exp_mul(qh, kc):
                    sp = [
                        ps_s.tile(
                            [128, 1024], f32, tag="s", name=f"sp_{kc}_{qh}_{hp2}"
                        )
                        for hp2 in range(2)
                    ]
                    for h in range(HPG):
                        hp = slice(32 * h, 32 * h + 32)
                        nc.tensor.matmul(
                            sp[h // 2][:, bass.ts(h % 2, 512)],
                            k_sb[hp, bass.ts(kc, 128)],
                            q_sb[hp, bass.ts(qh, 512)],
                            start=True,
                            stop=True,
                            tile_position=(32 * h, 0),
                            skip_group_check=True,
                        )
                    e2s = []
                    for hp2 in range(2):
                        es = es_p.tile(
                            [128, 1024], f16, tag="es", name=f"es_{kc}_{qh}_{hp2}"
                        )
                        nc.scalar.activation(
                            es[:], sp[hp2][:], AF.Exp, bias=mask[:, kc : kc + 1]
                        )
                        e2 = e2_p.tile(
                            [128, 1024], f16, tag="e2", name=f"e2_{kc}_{qh}_{hp2}"
                        )
                        nc.vector.tensor_mul(e2[:], es[:], pair_t[(qh, kc, hp2)][:])
                        e2s.append(e2)
                    e2_t[(qh, kc)] = e2s

                def av_rowsum(qh, kc):
                    if kc == 0:
                        o_ps[qh] = ps_o.tile(
                            [128, 512], f32, tag="o", name=f"o_ps{qh}"
                        )
                        r_ps[qh] = ps_r.tile(
                            [128, 512], f32, tag="r", name=f"r_ps{qh}"
                        )
                    e2s = e2_t.pop((qh, kc))
                    for h in range(HPG):
                        hp = slice(32 * h, 32 * h + 32)
                        nc.tensor.matmul(
                            o_ps[qh][hp, :],
                            v_sb[kc][:, hp],
                            e2s[h // 2][:, bass.ts(h % 2, 512)],
                            start=(kc == 0),
                            stop=(kc == KT - 1),
                            tile_position=(0, 32 * h),
                            skip_group_check=True,
                        )
                    for h in range(HPG):
                        hp = slice(32 * h, 32 * h + 32)
                        nc.tensor.matmul(
                            r_ps[qh][hp, :],
                            ones[:],
                            e2s[h // 2][:, bass.ts(h % 2, 512)],
                            start=(kc == 0),
                            stop=(kc == KT - 1),
                            tile_position=(0, 32 * h),
                            skip_group_check=True,
                        )

                def normalize(qh):
                    recip = nrm_p.tile([128, 512], f32, tag="recip", name="recip")
                    nc.vector.reciprocal_approx_fast(recip[:], r_ps[qh][:])
                    geff = nrm_p.tile([128, 512], f32, tag="geff", name="geff")
                    nc.vector.tensor_mul(geff[:], g_sb[:, bass.ts(qh, 512)], recip[:])
                    nc.vector.tensor_mul(
                        o_eff[:, bass.ts(qh, 512)], o_ps[qh][:], geff[:]
                    )

                def outproj(qh):
                    for qt in range(4 * qh, 4 * qh + 4):
                        ps = ps_s.tile([128, 256], f32, tag="s", name="ps_out")
                        nc.tensor.matmul(
                            ps[:],
                            o_eff[:, bass.ts(qt, 128)],
                            wo[:],
                            start=True,
                            stop=True,
                        )
                        ot = out_p.tile([128, 256], f32, tag="ot", name="ot")
                        nc.vector.tensor_add(ot[:], ps[:], bo[:])
                        nc.sync.dma_start(d_out[bass.ts(qt, 128), :], ot[:])

                # AV/rowsum lags QK/exp/mul by LAG units so its DVE input
                # (the e2 multiply) is always ready when the PE reaches it:
                # the in-order PE queue then never waits on the DVE, which
                # would otherwise recirculate a bubble through the
                # QK->exp->mul->AV loop.
                LAG = 1
                units = [(0, kc) for kc in range(KT)] + [
                    (1, kc) for kc in range(KT)
                ]
                for i, (qh, kc) in enumerate(units):
                    qk_exp_mul(qh, kc)
                    if i < LAG:
                        continue
                    pqh, pkc = units[i - LAG]
                    if (pqh, pkc) == (1, 0):
                        normalize(0)  # frees o/r banks before av_rowsum(1,0)
                    elif (pqh, pkc) == (1, 1):
                        outproj(0)
                    av_rowsum(pqh, pkc)
                for pqh, pkc in units[-LAG:]:
                    av_rowsum(pqh, pkc)
                normalize(1)
                outproj(1)

    nc.compile()
    return nc


_NC_CACHE = None


def _get_program():
    global _NC_CACHE
    if _NC_CACHE is None:
        _NC_CACHE = _build_program()
    return _NC_CACHE


def _shard_inputs(q_x, kv_x, bias_mask, bias_pair, Wq, Wk, Wv, Wo, bo, Wg, bg):
    """Build the 8 per-core input maps."""
    f = np.float32
    f16 = np.float16
    scale = 1.0 / math.sqrt(D)

    def fold2(w_t):  # [256, M] -> [128, 2*M] sbuf layout
        return np.ascontiguousarray(
            w_t.reshape(2, 128, w_t.shape[1]).transpose(1, 0, 2).reshape(128, -1)
        )

    in_maps = []
    for core in range(NCORES):
        b, hg = core // HG, core % HG
        hs = slice(hg * 128, hg * 128 + 128)  # H*D slice for this head group
        qxT = np.ascontiguousarray(q_x[b].T).astype(f)  # [256, 1024]
        kvxT = np.ascontiguousarray(kv_x[b].T).astype(f)
        # exp(pair) tiles: [qh, kc, hp2, 128, Q]; tile (qh,kc,hp2) holds heads
        # (2*hp2, 2*hp2+1) side by side for q-half qh, k-tile kc (transposed
        # to [k, q] to match the S.T layout).
        epT = np.exp(
            bias_pair[b, hg * HPG : hg * HPG + HPG].astype(f)
        ).transpose(0, 2, 1)  # [4, K, Q]
        ep = (
            epT.reshape(HPG, KT, 128, 2, 512)
            .transpose(3, 1, 0, 2, 4)  # [qh, kc, h, 128, 512]
            .reshape(2, KT, 2, 2, 128, 512)
            .transpose(0, 1, 2, 4, 3, 5)  # [qh, kc, hp2, 128, hi, 512]
            .reshape(2, KT, 2, 128, Q)
        )
        w4 = np.concatenate(
            [
                fold2(np.ascontiguousarray(Wq[hs].T) * scale),
                fold2(np.ascontiguousarray(Wk[hs].T)),
                fold2(np.ascontiguousarray(Wv[hs].T)),
                fold2(np.ascontiguousarray(Wg[hs].T)),
            ],
            axis=1,
        )
        sm16 = np.concatenate(
            [np.ones((128, 32), f), np.ascontiguousarray(Wo[:, hs].T)], axis=1
        )
        bo_b = (
            np.broadcast_to(bo, (128, C)).astype(f)
            if hg == 0
            else np.zeros((128, C), f)
        )
        sm32 = np.concatenate(
            [
                np.ascontiguousarray(bg[hs].reshape(128, 1)).astype(f),
                np.ascontiguousarray(
                    bias_mask[b, 0, 0].reshape(KT, 128).T - SHIFT
                ).astype(f),
                bo_b,
            ],
            axis=1,
        )
        m16 = {
            "qx": fold2(qxT),
            "kvx": fold2(kvxT),
            "ep": ep,
            "w4": w4,
            "sm16": sm16,
        }
        m = {k: np.ascontiguousarray(v, f16) for k, v in m16.items()}
        m["sm32"] = np.ascontiguousarray(sm32, f)
        in_maps.append(m)
    return in_maps


def run_on_cores(in_maps, trace=False, trace_kwargs={}):
    from concourse.bass_utils import run_bass_kernel_spmd

    nc = _get_program()
    return run_bass_kernel_spmd(
        nc, in_maps, list(range(NCORES)), trace=trace, trace_kwargs=trace_kwargs
    )


def kernel(q_x, kv_x, bias_mask, bias_pair, Wq, Wk, Wv, Wo, bo, Wg, bg):
    in_maps = _shard_inputs(
        q_x, kv_x, bias_mask, bias_pair, Wq, Wk, Wv, Wo, bo, Wg, bg
    )
    res = run_on_cores(in_maps).results
    out = np.empty((B, Q, C), np.float32)
    for b in range(B):
        out[b] = res[b * HG + 0]["out"] + res[b * HG + 1]["out"]
    return out
